# revision 2
# baseline (speedup 1.0000x reference)
"""ConditionalConv Trainium2 kernel (8 NeuronCores, SPMD) — v2.

Reference computation (per sample b):
    w_b = tanh(conditioning @ W_cond.T + b_cond) * 5        [B, 36928]
    bias = w_b[:, -64:]; w = w_b[:, :-64].reshape(B, 64, 64, 3, 3)
    y[b] = conv2d(x[b], w[b], pad=1) + bias[b]

v2 changes vs baseline:
  - gamma-packed conv matmuls: lhsT [128=(row-shift d, ic), 128=(half g, oc)]
    where half A = taps kh=d (kh 0,1) contributing to out rows h0+dh, and
    half B = tap kh=2 (d=0 rows; d=1 rows are zero) contributing to out rows
    h0+dh-2.  One N=512 matmul per kw per 4-row block covers all 3 kh taps:
    3 matmuls/block instead of 6 -> PE conv time ~89us -> ~42us.
  - Evacuation: walrus only allows ONE psum operand per vector op and Pool
    cannot read psum at all, so ACT stages gB+bias (psum hi -> sbuf hi,
    aligned) and DVE adds gA (a psum operand at a different base partition
    is allowed - verified on HW): y = (gB + 5*tanh_bias) + gA.
    The x5 scale is folded into the hypernet output (tanh then *5 on DVE),
    so conv weights and bias arrive pre-scaled.
  - hyp_out/hyp_rcv in fp16: halves AllToAll and weight-load traffic; the
    weight loads move to HWDGE (sync.dma) instead of Pool SWDGE since no
    f32->f16 cast is needed anymore.
  - x staged once from HBM per sample; the row-shifted upper 64 partitions
    are filled by an SBUF->SBUF DMA (halves HBM read traffic for x).
"""

import numpy as np
from contextlib import ExitStack

import concourse.bacc as bacc
import concourse.tile as tile
import concourse.mybir as mybir
from concourse.bass_utils import run_bass_kernel_spmd

dt = mybir.dt
AF = mybir.ActivationFunctionType
ALU = mybir.AluOpType

N_CORES = 8
B, COND_C = 16, 256
IN_C, OUT_C, KS = 64, 64, 3
H = W = 128
NW = KS * KS * IN_C * OUT_C          # 36864 weight params
N_PARAM = NW + OUT_C                 # 36928
SLICE = NW // N_CORES                # 4608 params per core
HSN = SLICE + OUT_C                  # 4672 hypernet outputs per core
S = B // N_CORES                     # 2 samples per core
HP = H + 2                           # 130 padded
NBLK = H // 4                        # 32 full 4-row conv blocks
UNROLL = 4                           # logical iterations per For_i body (timing)
KCH = 3                              # hypernet contraction chunks of <=128

_cache = {}


def _build(loop=0, mode="full"):
    """Build + compile the 8-core SPMD bass program.

    mode (timing experiments only; 'full' for the real kernel):
      'empty': both loops contain a single tiny memset.
      'dma':   loop2 = x-input DMAs only; loop1 = memset.
      'conv':  loop2 = wload+conv, no x loads; loop1 = memset.
    """
    nc = bacc.Bacc("TRN2", target_bir_lowering=False, debug=False,
                   num_devices=N_CORES)

    xs = nc.dram_tensor("xs", [S, IN_C, HP, HP], dt.float16, kind="ExternalInput").ap()
    hs = nc.dram_tensor("hs", [COND_C + 1, HSN], dt.float16, kind="ExternalInput").ap()
    ct = nc.dram_tensor("ct", [COND_C + 1, B], dt.float16, kind="ExternalInput").ap()
    ys = nc.dram_tensor("ys", [S, OUT_C, H, W], dt.float32, kind="ExternalOutput").ap()

    hyp_out = nc.dram_tensor("hyp_out", [B, HSN], dt.float16, kind="Internal")
    hyp_rcv = nc.dram_tensor("hyp_rcv", [B, HSN], dt.float16, kind="Internal")

    with tile.TileContext(nc) as tc:
        with ExitStack() as ctx:
            cpool = ctx.enter_context(tc.tile_pool(name="consts", bufs=1))
            hpool = ctx.enter_context(tc.tile_pool(name="hyp", bufs=3))
            epool = ctx.enter_context(tc.tile_pool(name="evac", bufs=2))
            ppool = ctx.enter_context(tc.tile_pool(name="psum", bufs=2, space="PSUM"))

            # ---------------- persistent tile allocs ----------------
            # Hypernet K chunks of 86/86/85 (all round up to the 128-row PE
            # tiling mode, so the 257th ones-row costs nothing extra).
            CH = [(0, 86), (86, 172), (172, COND_C + 1)]
            cts = [cpool.tile([128, B], dt.float16, name=f"ct{k}")
                   for k in range(KCH)]
            hss = [cpool.tile([128, HSN], dt.float16, name=f"hs{k}")
                   for k in range(KCH)]
            # Per-sample input: partitions (d, ic) with d in {0,1}; the upper
            # half holds the image shifted down one row so one K=128
            # contraction sees xpad rows r and r+1.
            xdup = [cpool.tile([128, HP * HP], dt.float16, name=f"xdup{s}")
                    for s in range(S)]
            xdv = [xdup[s][:].rearrange("p (h w) -> p h w", w=HP)
                   for s in range(S)]
            # gamma-packed conv weights, one [128,128] tile per (sample, kw).
            wt = [[cpool.tile([128, 128], dt.float16, name=f"wt{s}_{kw}")
                   for kw in range(KS)] for s in range(S)]
            # conv bias (already 5*tanh(b), fp16 in hyp_rcv) -> fp32, staged
            # in partitions 64:128 to align with the gB psum half.
            tb16 = [cpool.tile([128, 1], dt.float16, name=f"tb16_{s}")
                    for s in range(S)]
            tb = [cpool.tile([128, 1], dt.float32, name=f"tb{s}")
                  for s in range(S)]

            # One-time zero init (outside the timing loops): the (d=1, gB)
            # weight quarter, and the tail of the shifted x copy (rows beyond
            # the image would otherwise be uninitialised SBUF; 0*NaN = NaN in
            # the PE would poison the gB accumulation).
            for s in range(S):
                for kw in range(KS):
                    nc.vector.memset(wt[s][kw][64:128, 64:128], 0.0)
                nc.vector.memset(xdup[s][64:128, (HP - 1) * HP:], 0.0)

            dummy = cpool.tile([128, 512], dt.float32, name="dummy")

            loop_cm = (tc.For_i(0, loop, 1,
                                hint_engines=(mybir.EngineType.PE,))
                       if loop else None)
            if loop_cm is not None:
                loop_cm.__enter__()

            if mode != "full":
                nc.gpsimd.memset(dummy[:, 0:8], 0.0)

            def phase1(u):
                # -------------- input loads (phase 1) --------------
                for k, (klo, khi) in enumerate(CH):
                    if mode not in ("full",) and loop:
                        break
                    nc.sync.dma_start(cts[k][0:khi - klo, :], ct[klo:khi, :])
                    nc.sync.dma_start(hss[k][0:khi - klo, :], hs[klo:khi, :])
                # -------------- hypernetwork --------------
                # out[b, p] = sum_c cond[b, c]*Wp[p, c] (+ b_cond via ones
                # row); tanh on ACT during PSUM evacuation, then *5 + fp16
                # cast on DVE.
                for j in range(0 if (mode != "full" and loop) else 10):
                    n0 = j * 512
                    nn = 512 if j < 9 else OUT_C
                    hp = ppool.tile([B, nn], dt.float32, name=f"hp{u}_{j}",
                                    tag=f"acc{j % 4}")
                    for k, (klo, khi) in enumerate(CH):
                        nc.tensor.matmul(hp[:], cts[k][0:khi - klo, :],
                                         hss[k][0:khi - klo, n0:n0 + nn],
                                         start=(k == 0), stop=(k == KCH - 1))
                    tht = hpool.tile([B, nn], dt.float32, name=f"th{u}_{j}",
                                     tag="th")
                    th5 = hpool.tile([B, nn], dt.float16, name=f"t5{u}_{j}",
                                     tag="t5")
                    nc.scalar.activation(tht[:], hp[:], AF.Tanh)
                    nc.vector.tensor_scalar_mul(th5[:], tht[:], 5.0)
                    nc.sync.dma_start(hyp_out.ap()[:, n0:n0 + nn], th5[:])

            with nc.named_scope("hyper"):
                for u in range(UNROLL if loop else 1):
                    phase1(u)

            if loop_cm is not None:
                loop_cm.__exit__(None, None, None)

            # ---------------- redistribute ----------------
            with nc.named_scope("cc"):
                nc.gpsimd.collective_compute(
                    "AllToAll", ALU.bypass,
                    replica_groups=[list(range(N_CORES))],
                    ins=[hyp_out.ap()], outs=[hyp_rcv.ap()],
                )

            loop_cm2 = (tc.For_i(0, loop, 1,
                                 hint_engines=(mybir.EngineType.PE,))
                        if loop else None)
            if loop_cm2 is not None:
                loop_cm2.__enter__()

            # ---------------- input loads (phase 2) ----------------
            # HBM load of the lower half, then an SB->SB DMA for the
            # one-row-down shifted upper half.  Chunked by row ranges so a
            # following unrolled iteration's loads can overlap the previous
            # iteration's conv tail (subtile deps).
            XCH = [r * HP for r in (0, 33, 66, 99, HP)]

            def phase2_loads(u):
                if mode == "empty" and loop:
                    nc.gpsimd.memset(dummy[:, 8:16], 0.0)
                for s in range(S):
                    if mode in ("empty", "conv") and loop:
                        break
                    xsf = xs[s].rearrange("c h w -> c (h w)")
                    for a, b_ in zip(XCH[:-1], XCH[1:]):
                        nc.sync.dma_start(xdup[s][0:64, a * 1:b_ * 1],
                                          xsf[:, a:b_])
                    for a, b_ in zip(XCH[:-1], XCH[1:]):
                        b2 = min(b_, (HP - 1) * HP)
                        nc.sync.dma_start(xdup[s][64:128, a:b2],
                                          xdup[s][0:64, a + HP:b2 + HP])

            # ---------------- conv weight tiles ----------------
            # hyp_rcv row k*S+s = (my sample s)'s params [k*4608, (k+1)*4608),
            # fp16, already 5*tanh.  Permuted param index n = tap*4096 +
            # ic*64 + oc.  lhsT quarters (partition rows x cols):
            #   kh=0 -> [0:64,   0:64]   (d=0, gA)
            #   kh=1 -> [64:128, 0:64]   (d=1, gA)
            #   kh=2 -> [0:64,  64:128]  (d=0, gB); [64:128, 64:128] is zero.
            hv = hyp_rcv.ap().rearrange("b (p q) -> b p q", q=64)  # [16, 73, 64]

            def wload(u):
                for t in range(0 if (mode in ("empty", "dma") and loop) else KS * KS):
                    kh, kw = divmod(t, KS)
                    lo, hi = t * 4096, (t + 1) * 4096
                    cuts = [lo] + [m for m in range(SLICE, NW, SLICE)
                                   if lo < m < hi] + [hi]
                    prow = 0 if kh != 1 else 64
                    pcol = 0 if kh != 2 else 64
                    for s in range(S):
                        for a, b_ in zip(cuts[:-1], cuts[1:]):
                            k = a // SLICE
                            src = hv[k * S + s,
                                     (a - k * SLICE) // 64:(b_ - k * SLICE) // 64, :]
                            dst = wt[s][kw][prow + (a - lo) // 64:
                                            prow + (b_ - lo) // 64,
                                            pcol:pcol + 64]
                            nc.sync.dma_start(dst, src)
                # conv bias (5*tanh(b)) fp16 -> fp32 column vectors @ parts 64+
                for s in range(S):
                    if mode in ("empty", "dma") and loop:
                        break
                    nc.sync.dma_start(tb16[s][64:128, :],
                                      hyp_rcv.ap()[s, SLICE:SLICE + OUT_C])
                    nc.vector.tensor_copy(tb[s][64:128, :], tb16[s][64:128, :])

            # ---------------- conv ----------------
            # Block j (h0 = 4j): psum P[128, 512]:
            #   parts 0:64   (gA) = out rows 4j..4j+3, taps kh=0,1
            #   parts 64:128 (gB) = out rows 4j-2..4j+1, tap kh=2
            # Extra partial block j=32 (h0=128, N=256) supplies gB for rows
            # 126/127.  Evac:
            #   ACT: gb[j] = P_j[64:128, :] + 5*tanh_bias      (aligned @64+)
            #   DVE: y[4j..4j+1]   = gb[j][256:512]   + P_j[0:64, 0:256]
            #        y[4j+2..4j+3] = gb[j+1][0:256]   + P_j[0:64, 256:512]
            # (one psum operand per DVE op; its base-partition mismatch vs
            # the sbuf operands is allowed for PSUM - HW-verified.)
            ysv4 = ys.rearrange("s c (j v) w -> s c j (v w)", v=4)  # [S,64,32,512]

            def conv(u):
                P = [[None] * (NBLK + 1) for _ in range(S)]
                gb = [[None] * (NBLK + 1) for _ in range(S)]
                yst = [[None] * NBLK for _ in range(S)]
                for j in range(0 if (mode in ("empty", "dma") and loop) else NBLK + 1):
                    ncols = 512 if j < NBLK else 256
                    nrow = 4 if j < NBLK else 2
                    for s in range(S):
                        P[s][j] = ppool.tile([128, 512], dt.float32,
                                             name=f"P{u}_{s}_{j}",
                                             tag=f"acc{2 * s + j % 2}")
                        for kw in range(KS):
                            nc.tensor.matmul(
                                P[s][j][:, 0:ncols], wt[s][kw][:, :],
                                xdv[s][:, 4 * j:4 * j + nrow, kw:kw + 128],
                                start=(kw == 0), stop=(kw == KS - 1))
                    for s in range(S):
                        gb[s][j] = epool.tile([128, 512], dt.float32,
                                              name=f"gb{u}_{s}_{j}",
                                              tag=f"gb{s}")
                        nc.scalar.activation(gb[s][j][64:128, 0:ncols],
                                             P[s][j][64:128, 0:ncols],
                                             AF.Identity,
                                             bias=tb[s][64:128, :])
                        if j >= 1:
                            nc.vector.tensor_add(yst[s][j - 1][64:128, 256:512],
                                                 gb[s][j][64:128, 0:256],
                                                 P[s][j - 1][0:64, 256:512])
                            nc.sync.dma_start(ysv4[s, :, j - 1, :],
                                              yst[s][j - 1][64:128, :])
                        if j < NBLK:
                            yst[s][j] = epool.tile([128, 512], dt.float32,
                                                   name=f"yt{u}_{s}_{j}",
                                                   tag=f"ys{s}{j % 3}")
                            nc.vector.tensor_add(yst[s][j][64:128, 0:256],
                                                 gb[s][j][64:128, 256:512],
                                                 P[s][j][0:64, 0:256])

            with nc.named_scope("conv"):
                for u in range(UNROLL if loop else 1):
                    phase2_loads(u)
                    wload(u)
                    conv(u)

            if loop_cm2 is not None:
                loop_cm2.__exit__(None, None, None)

    nc.compile()
    return nc


def _prep_inputs(x, conditioning, W_cond, b_cond):
    """Host-side shard + permute. Returns per-core input maps."""
    x = np.asarray(x, dtype=np.float32)
    conditioning = np.asarray(conditioning, dtype=np.float32)
    W_cond = np.asarray(W_cond, dtype=np.float32)
    b_cond = np.asarray(b_cond, dtype=np.float32)

    t = np.arange(KS * KS)
    i = np.arange(IN_C)
    o = np.arange(OUT_C)
    # permuted n = (tap, ic, oc) -> original p = oc*576 + ic*9 + tap
    perm = (o[None, None, :] * (IN_C * KS * KS) + i[None, :, None] * (KS * KS)
            + t[:, None, None]).reshape(-1)
    Wp = W_cond[perm]                      # [36864, 256]
    bp = b_cond[perm]

    # [257, 36864]: rows 0-255 = Wp^T, row 256 = bp (ones-row bias fold)
    AaugW = np.zeros((COND_C + 1, NW), np.float16)
    AaugW[0:COND_C] = Wp.T.astype(np.float16)
    AaugW[COND_C] = bp.astype(np.float16)
    AaugB = np.zeros((COND_C + 1, OUT_C), np.float16)
    AaugB[0:COND_C] = W_cond[NW:].T.astype(np.float16)
    AaugB[COND_C] = b_cond[NW:].astype(np.float16)

    ctaug = np.zeros((COND_C + 1, B), np.float16)
    ctaug[0:COND_C] = conditioning.T.astype(np.float16)
    ctaug[COND_C] = 1.0

    xpadded = np.zeros((B, IN_C, HP, HP), np.float16)
    xpadded[:, :, 1:HP - 1, 1:HP - 1] = x.astype(np.float16)

    in_maps = []
    for c in range(N_CORES):
        hs_c = np.ascontiguousarray(
            np.concatenate([AaugW[:, c * SLICE:(c + 1) * SLICE], AaugB], axis=1),
            dtype=np.float16)
        xs_c = np.ascontiguousarray(xpadded[c * S:(c + 1) * S])  # float16
        in_maps.append({"xs": xs_c, "hs": hs_c, "ct": ctaug})
    return in_maps


def _get_nc(loop=0, mode="full"):
    key = (loop, mode)
    if key not in _cache:
        _cache[key] = _build(loop, mode)
    return _cache[key]


def _assemble(results):
    return np.concatenate([results[c]["ys"] for c in range(N_CORES)], axis=0)


def kernel(x, conditioning, W_cond, b_cond):
    nc = _get_nc()
    in_maps = _prep_inputs(x, conditioning, W_cond, b_cond)
    res = run_bass_kernel_spmd(nc, in_maps, list(range(N_CORES)))
    return _assemble(res.results)


# ---- helpers for the local test harness (not used by the grader) ----

def run_sim(x, conditioning, W_cond, b_cond):
    import concourse.bass_interp as bass_interp

    nc = _get_nc()
    in_maps = _prep_inputs(x, conditioning, W_cond, b_cond)
    sim = bass_interp.MultiCoreSim(nc, N_CORES)
    for c in range(N_CORES):
        for k, v in in_maps[c].items():
            sim.cores[c].tensor(k)[:] = v
    sim.simulate()
    results = [{"ys": np.array(sim.cores[c].tensor("ys"))} for c in range(N_CORES)]
    return _assemble(results)
